# revision 1
# baseline (speedup 1.0000x reference)
"""Equivariant-subsample (shifted 2x2 max-pool) Trainium2 kernel.

Problem: images [16,64,512,512] f32, per-(b,c) offsets p_h, p_w in {0,1}.
out[b,c,i,j] = max over rows {rA, rA+1} x cols {cA, cA+1} of images[b,c]
where rA = min(2*i + p_h, 510), cA = min(2*j + p_w, 510).

Strategy (8 NeuronCores, embarrassingly data-parallel):
  - Flatten (b,c) -> 1024 images; core k owns images [k*128, (k+1)*128).
    One image per SBUF partition.
  - Key observation: the two source rows of every output row are ADJACENT
    (rowB = rowA + 1 even at the clamp), so a single gather index per
    output row fetches a contiguous 1024-element (4 KB) segment covering
    both rows, with the p_w column shift folded into the element offset.
    Indices are computed on the host from p_h/p_w (tiny metadata, like DMA
    descriptors) and uploaded as an int32 tensor; the compiled program is
    input-independent.
  - On device: indirect DMA gather (SWDGE) -> 4-way strided tensor_tensor
    max on DVE (a pure 2x2 maxpool) -> store.  The only place the p_w
    clamp deviates from the uniform stride is output column 255; a tiny
    per-partition blend (biases uploaded from host) fixes it.
  - Out-of-bounds tail: a segment of the last row with p_w=1 spills 1
    element past the image.  Cores 0-6 use an input view overlapping the
    next core's first row (zero-copy); core 7 gets a 2 KB zero pad.
"""

import sys

import numpy as np

sys.path.insert(0, "/opt/trn_rl_repo")

B, C, H, W = 16, 64, 512, 512
HR = WR = 2
OH, OW = H // HR, W // WR
NCORES = 8
P = 128                     # SBUF partitions == images per core
IMGS = (B * C) // NCORES    # 128
RC = 8                      # output rows per chunk
NCHUNK = OH // RC
NROWS_PAD = IMGS * H + 2    # input rows per core incl. 2 pad rows
NEG = np.float32(-3.0e38)

_prog = None


def _legalize_waits(nc, mybir, dummy_sem_id, dummy_sem_name):
    """Split multi-wait instructions: this walrus build encodes only ONE
    sync-wait per engine/DMA instruction.  Hoist extra waits onto no-op
    instructions inserted just before, on the same engine (the sequencer
    executes them in order, so the AND-semantics are preserved)."""
    for fn in nc.m.functions:
        for blk in fn.blocks:
            new_insts = []
            for inst in blk.instructions:
                si = getattr(inst, "sync_info", None)
                if si is not None and si.on_wait and len(si.on_wait) > 1:
                    for w in si.on_wait[:-1]:
                        nop = mybir.InstNoOp(
                            name=nc.get_next_instruction_name(),
                            engine=inst.engine,
                            text_hint="wait_split",
                            bass_nofuse=True,
                        )
                        # +1 update on a dedicated, never-waited semaphore
                        # keeps the race detector and ISA checks happy
                        # without perturbing any real threshold.
                        nop.sync_info = mybir.SyncInfo(
                            on_wait=[w],
                            on_update=[
                                mybir.SyncUpdate(
                                    sync_type="semaphore",
                                    id=dummy_sem_id,
                                    update_mode="sem-inc",
                                    ant_name=dummy_sem_name,
                                    update_value=1,
                                )
                            ],
                        )
                        new_insts.append(nop)
                    si.on_wait = si.on_wait[-1:]
                new_insts.append(inst)
            blk.instructions = new_insts


def _build_program():
    from concourse import bass, mybir
    import concourse.tile as tile

    f32 = mybir.dt.float32
    i32 = mybir.dt.int32

    nc = bass.Bass()
    legal_sem = nc.alloc_semaphore("legalize_nop")
    img = nc.declare_dram_parameter("img", [NROWS_PAD, W], f32, isOutput=False)
    # idx[:, :NCHUNK]: one gather index per (partition, chunk) — each
    # partition's chunk of 2*RC input rows is contiguous in DRAM, so one
    # 32 KB descriptor per partition replaces 8 4 KB ones (per-descriptor
    # overhead halves SDMA throughput otherwise).  idx[:, NCHUNK]: the
    # (510,511) edge-pair index.
    idx = nc.declare_dram_parameter("idx", [P, NCHUNK + 1], i32, isOutput=False)
    bias = nc.declare_dram_parameter("bias", [P, 2], f32, isOutput=False)
    out = nc.declare_dram_parameter("out", [P, OH * OW], f32, isOutput=True)

    with tile.TileContext(nc) as tc:
        with (
            tc.tile_pool(name="const", bufs=1) as cpool,
            tc.tile_pool(name="ld", bufs=1) as ldpool,
            tc.tile_pool(name="work", bufs=1) as wpool,
            tc.tile_pool(name="res", bufs=1) as rpool,
        ):
            idx_sb = cpool.tile([P, NCHUNK + 1], i32)
            nc.sync.dma_start(out=idx_sb[:], in_=idx[:])
            bias_sb = cpool.tile([P, 2], f32)
            nc.sync.dma_start(out=bias_sb[:], in_=bias[:])
            # Output row 255 is parity-independent (always source rows
            # 510/511): compute it from a dedicated one-index-per-partition
            # gather of that row pair, shifted by pw.
            et = cpool.tile([P, 2 * W], f32)
            nc.gpsimd.indirect_dma_start(
                out=et[:],
                out_offset=None,
                in_=img[:],
                in_offset=bass.IndirectOffsetOnAxis(
                    ap=idx_sb[:, NCHUNK:NCHUNK + 1], axis=1
                ),
            )
            ev = et[:].rearrange("p (a j e) -> p a j e", a=2, j=OW, e=2)
            eu1 = cpool.tile([P, OW], f32)
            eu2 = cpool.tile([P, OW], f32)
            ea = cpool.tile([P, OW], f32)
            nc.vector.tensor_tensor(
                out=eu1[:], in0=ev[:, 0, :, 0], in1=ev[:, 1, :, 0],
                op=mybir.AluOpType.max,
            )
            nc.vector.tensor_tensor(
                out=eu2[:], in0=ev[:, 0, :, 1], in1=ev[:, 1, :, 1],
                op=mybir.AluOpType.max,
            )
            nc.vector.tensor_tensor(
                out=ea[:], in0=eu1[:], in1=eu2[:], op=mybir.AluOpType.max,
            )
            ew = et[:].rearrange("p (a w) -> p a w", a=2, w=W)
            ee2 = cpool.tile([P, 2], f32)
            nc.vector.tensor_tensor(
                out=ee2[:], in0=ew[:, 0, 509:511], in1=ew[:, 1, 509:511],
                op=mybir.AluOpType.max,
            )
            efx = cpool.tile([P, 1], f32)
            nc.vector.tensor_tensor(
                out=efx[:], in0=ee2[:, 0:1], in1=ee2[:, 1:2],
                op=mybir.AluOpType.max,
            )
            eta = cpool.tile([P, 1], f32)
            etb = cpool.tile([P, 1], f32)
            nc.vector.tensor_tensor(
                out=eta[:], in0=ea[:, 255:256], in1=bias_sb[:, 0:1],
                op=mybir.AluOpType.add,
            )
            nc.vector.tensor_tensor(
                out=etb[:], in0=efx[:], in1=bias_sb[:, 1:2],
                op=mybir.AluOpType.add,
            )
            nc.vector.tensor_tensor(
                out=ea[:, 255:256], in0=eta[:], in1=etb[:],
                op=mybir.AluOpType.max,
            )

            for c in range(NCHUNK):
                # Explicit modular tags force true round-robin slot reuse:
                # the TileScheduler's allocator otherwise reuses the most
                # recently freed slot (LIFO), collapsing the pipeline to
                # depth 1.
                # Gather the chunk: 2*RC contiguous input rows per partition,
                # shifted by (ph, pw) via the per-partition index.
                ld = ldpool.tile([P, RC * 2 * W], f32, tag=f"ld{c % 3}")
                nc.gpsimd.indirect_dma_start(
                    out=ld[:],
                    out_offset=None,
                    in_=img[:],
                    in_offset=bass.IndirectOffsetOnAxis(
                        ap=idx_sb[:, c:c + 1], axis=1
                    ),
                )

                # ld[p, ((r*2 + a)*OW + j)*2 + e] = row a of pair r, col 2j+e
                ldv = ld[:].rearrange("p (r a j e) -> p r a j e", a=2, j=OW, e=2)
                t1 = wpool.tile([P, RC * OW], f32, tag=f"t1_{c % 2}")
                t2 = wpool.tile([P, RC * OW], f32, tag=f"t2_{c % 2}")
                oc = rpool.tile([P, RC * OW], f32, tag=f"oc{c % 3}")
                # Claim the output slot with a 1-elem DVE write: it absorbs
                # the WAR-vs-store DMA sem wait so the heavy ops after it
                # carry at most one wait each.
                nc.vector.tensor_copy(out=oc[0:1, 0:1], in_=bias_sb[0:1, 0:1])
                t1v = t1[:].rearrange("p (r j) -> p r j", j=OW)
                t2v = t2[:].rearrange("p (r j) -> p r j", j=OW)
                ocv = oc[:].rearrange("p (r j) -> p r j", j=OW)
                # 2x2 max pool: max over row-in-pair (a) and col-in-pair (e)
                nc.vector.tensor_tensor(
                    out=t1v, in0=ldv[:, :, 0, :, 0], in1=ldv[:, :, 1, :, 0],
                    op=mybir.AluOpType.max,
                )
                nc.vector.tensor_tensor(
                    out=t2v, in0=ldv[:, :, 0, :, 1], in1=ldv[:, :, 1, :, 1],
                    op=mybir.AluOpType.max,
                )
                nc.vector.tensor_tensor(
                    out=ocv, in0=t1v, in1=t2v, op=mybir.AluOpType.max,
                )

                # Output col 255 fix: when pw==1 the correct value is the max
                # over segment positions (509, 510) of both rows (= source
                # cols 510, 511); the uniform stride used (510, 511) instead.
                ldw = ld[:].rearrange("p (r a w) -> p r a w", a=2, w=W)
                e2 = wpool.tile([P, RC * 2], f32, tag=f"e2_{c % 2}")
                e2v = e2[:].rearrange("p (r e) -> p r e", e=2)
                nc.vector.tensor_tensor(
                    out=e2v, in0=ldw[:, :, 0, 509:511], in1=ldw[:, :, 1, 509:511],
                    op=mybir.AluOpType.max,
                )
                fx = wpool.tile([P, RC], f32, tag=f"fx_{c % 2}")
                nc.vector.tensor_tensor(
                    out=fx[:], in0=e2v[:, :, 0], in1=e2v[:, :, 1],
                    op=mybir.AluOpType.max,
                )
                ta = wpool.tile([P, RC], f32, tag=f"ta_{c % 2}")
                tb = wpool.tile([P, RC], f32, tag=f"tb_{c % 2}")
                # bias0 = (pw==0 ? 0 : -BIG), bias1 = (pw==0 ? -BIG : 0)
                nc.vector.tensor_tensor(
                    out=ta[:], in0=ocv[:, :, 255],
                    in1=bias_sb[:, 0:1].to_broadcast([P, RC]),
                    op=mybir.AluOpType.add,
                )
                nc.vector.tensor_tensor(
                    out=tb[:], in0=fx[:],
                    in1=bias_sb[:, 1:2].to_broadcast([P, RC]),
                    op=mybir.AluOpType.add,
                )
                nc.vector.tensor_tensor(
                    out=ocv[:, :, 255], in0=ta[:], in1=tb[:],
                    op=mybir.AluOpType.max,
                )

                if c == NCHUNK - 1:
                    # Replace the (garbage, unclamped-gather) last output row
                    # with the edge-pair result.
                    nc.vector.tensor_copy(out=ocv[:, RC - 1, :], in_=ea[:])

                nc.sync.dma_start(
                    out=out[:, c * RC * OW:(c + 1) * RC * OW], in_=oc[:]
                )
    _legalize_waits(nc, mybir, legal_sem.num, legal_sem.name)
    return nc


def _host_inputs(images, p_w, p_h):
    """Build the 8 per-core input maps (views wherever possible)."""
    flat = np.ascontiguousarray(images, dtype=np.float32).reshape(-1)
    ph = np.asarray(p_h).reshape(-1).astype(np.int64)
    pw = np.asarray(p_w).reshape(-1).astype(np.int64)
    nelem = IMGS * H * W
    i = np.arange(OH, dtype=np.int64)
    in_maps = []
    for k in range(NCORES):
        if k < NCORES - 1:
            img_k = flat[k * nelem:(k + 1) * nelem + 2 * W].reshape(NROWS_PAD, W)
        else:
            img_k = np.concatenate(
                [flat[k * nelem:], np.zeros(2 * W, np.float32)]
            ).reshape(NROWS_PAD, W)
        phk = ph[k * IMGS:(k + 1) * IMGS]
        pwk = pw[k * IMGS:(k + 1) * IMGS]
        # One index per chunk: the chunk's 2*RC input rows are contiguous in
        # DRAM (consecutive pairs are adjacent rows), so each partition's
        # chunk is a single 2*RC*W-element read starting at row 2*RC*c + ph,
        # col ph... shifted by pw.  Unclamped: the last chunk of a ph=1
        # image reads one garbage row; output row 255 is overwritten
        # on-device from the edge-pair gather (extra column).
        cidx = np.arange(NCHUNK, dtype=np.int64)
        base = np.arange(IMGS, dtype=np.int64)[:, None] * H
        idx_main = (base + 2 * RC * cidx[None, :] + phk[:, None]) * W + pwk[:, None]
        idx_edge = (base[:, 0] + H - HR) * W + pwk                   # [IMGS]
        idx = np.concatenate(
            [idx_main, idx_edge[:, None]], axis=1
        ).astype(np.int32)
        bias = np.stack(
            [np.where(pwk == 0, 0.0, NEG), np.where(pwk == 0, NEG, 0.0)],
            axis=1,
        ).astype(np.float32)
        in_maps.append({"img": img_k, "idx": idx, "bias": bias})
    return in_maps


def _get_prog():
    global _prog
    if _prog is None:
        _prog = _build_program()
    return _prog


def kernel(images, p_w, p_h, _return_raw=False, **run_kwargs):
    from concourse.bass_utils import run_bass_kernel_spmd

    in_maps = _host_inputs(images, p_w, p_h)
    res = run_bass_kernel_spmd(
        _get_prog(), in_maps, list(range(NCORES)), **run_kwargs
    )
    outs = [r["out"].reshape(IMGS, OH, OW) for r in res.results]
    full = np.concatenate(outs, axis=0).reshape(B, C, OH, OW)
    if _return_raw:
        return full, res
    return full

